# revision 18
# baseline (speedup 1.0000x reference)
"""Trainium2 Bass kernel for causal self-attention with RoPE.

Problem: B=2, T=2048, C=1024, H=16 heads, hd=64, fp32 in/out, causal, rotary.

Sharding: 8 cores = 2 batches x 4 head-groups. Core c handles batch c//4 and
heads [4*(c%4), 4*(c%4)+4). Each core computes its heads' Q/K/V projections,
RoPE, causal attention, and a partial output projection over its 256 input
channels; the host sums the 4 partial projections per batch and adds bp.

v2 design notes (from the v1 trace: ACT/exp is the co-bottleneck with PE,
and the v1 tail serialized on norm chains):
 - The two heads of a pair write adjacent PSUM banks of one [128,1024] tile
   and a single EXP covers both (1147ns vs 2x720ns); deep-diagonal rounds
   use two trimmed exps instead.
 - PSUM tags: pA/pB = two [128,1024] score tiles (2 banks each), y0..y3 =
   per-head attV accumulators (64 y-rows + z-row). Attention windows run
   per-pair so the inactive pair's y-banks serve as filler/proj PSUM.
 - Rope: ACT does the bias-add + fp32->fp16 cast out of PSUM (Identity with
   per-partition bias); sin is pre-shuffled on the host so DVE does only
   mul/shuffle/mul/add on fp16.
 - All output projections run as PE filler inside the ACT-bound score phases
   or the attV-w3 phase; out-DMA goes per 512-col chunk after each cast.
 - Input DMA: few big multi-dim transfers (v1 startup was sync-issue bound),
   ordered so V-units (cheapest deps) start first; issued from 4 engines.
 - Norm chains: reciprocal reads z directly from PSUM, gpsimd broadcasts,
   DVE multiplies; per-head chains are emitted stage-interleaved.
"""

import time
from collections import deque
from contextlib import ExitStack

import numpy as np

import concourse.bass as bass
import concourse.tile as tile
from concourse import bacc, library_config, mybir
from concourse.bass_utils import run_bass_kernel_spmd

F32 = mybir.dt.float32
F16 = mybir.dt.float16

T = 2048
C = 1024
HD = 64
NCORES = 8
NEG = -1e10
LAG = 3
SWAP_MASK = list(range(16, 32)) + list(range(16))

AF = mybir.ActivationFunctionType
ALU = mybir.AluOpType

LAST_EXEC_NS = None
LAST_RESULTS = None


def build_nc():
    nc = bacc.Bacc("TRN2", target_bir_lowering=False, debug=False)

    xT = nc.dram_tensor("xT", [C + 1, T], F16, kind="ExternalInput").ap()
    wqT = nc.dram_tensor("wqT", [C, 256], F16, kind="ExternalInput").ap()
    wkT = nc.dram_tensor("wkT", [C, 256], F16, kind="ExternalInput").ap()
    wvT = nc.dram_tensor("wvT", [C + 128, 256], F16, kind="ExternalInput").ap()
    wpT = nc.dram_tensor("wpT", [256, C], F16, kind="ExternalInput").ap()
    bqk = nc.dram_tensor("bqk", [128, 4], F32, kind="ExternalInput").ap()
    cc_d = nc.dram_tensor("cc", [128, T], F16, kind="ExternalInput").ap()
    ss_d = nc.dram_tensor("ss", [128, T], F16, kind="ExternalInput").ap()
    tri_d = nc.dram_tensor("tri", [128, 128], F32, kind="ExternalInput").ap()
    out_d = nc.dram_tensor("out", [T, C], F16, kind="ExternalOutput").ap()

    with tile.TileContext(nc) as tc, ExitStack() as ctx:
        consts = ctx.enter_context(tc.tile_pool(name="consts", bufs=1))

        cc_sb = consts.tile([128, T], F16)
        ss_sb = consts.tile([128, T], F16)   # pre-shuffled+signed sin
        tri_sb = consts.tile([128, 128], F32)
        bqk_sb = consts.tile([128, 4], F32)
        x1 = consts.tile([1, T], F16)

        # rotated Q^T / K^T: [pair][half] tiles (Q pairs 0-1, K pairs 2-3)
        qkt = [[consts.tile([128, 1024], F16, name=f"qkt{p}_{h}")
                for h in range(2)] for p in range(4)]
        vp = [consts.tile([128, 4 * 65], F16, name=f"vp{i}") for i in range(16)]
        vview = [v.rearrange("p (h d) -> p h d", d=65) for v in vp]
        usc = [[consts.tile([128, 512], F16, name=f"usc{p}_{w}")
                for w in range(4)] for p in range(2)]
        wp_sb = [consts.tile([128, C], F16, name=f"wp{p}") for p in range(2)]
        xts = [consts.tile([128, T], F16, name=f"xt{j}") for j in range(8)]
        # packed weights: 8 (9 for V) row-chunks side by side in the free dim
        wq_sb = consts.tile([128, 8 * 256], F16)
        wk_sb = consts.tile([128, 8 * 256], F16)
        wv_sb = consts.tile([128, 9 * 256], F16)

        h0, h1 = slice(0, 1024), slice(1024, 2048)

        # ---------- input DMA: few big transfers, priority order ----------
        wv_src = wvT.rearrange("(a p) c -> p a c", p=128)
        wq_src = wqT.rearrange("(a p) c -> p a c", p=128)
        wk_src = wkT.rearrange("(a p) c -> p a c", p=128)
        wp_src = wpT.rearrange("(a p) c -> p a c", p=128)

        # gpsimd: memsets first (they gate the V units), then the library;
        # no DMAs here — SWDGE issue costs ~1.7us each.
        nc.gpsimd.memset(x1[:], 1.0)
        for i in range(16):
            nc.gpsimd.memset(vview[i][:, :, 64], 1.0)
        nc.gpsimd.load_library(library_config.attn)

        q0, q1 = slice(0, 512), slice(512, 1024)
        nc.sync.dma_start(wv_sb.rearrange("p (a c) -> p a c", c=256), wv_src)
        nc.scalar.dma_start(bqk_sb[:], bqk[:])
        nc.scalar.dma_start(cc_sb[:, q0], cc_d[:, q0])
        nc.scalar.dma_start(ss_sb[:, q0], ss_d[:, q0])
        nc.scalar.dma_start(wq_sb.rearrange("p (a c) -> p a c", c=256),
                            wq_src)
        for j in range(8):
            nc.sync.dma_start(xts[j][:, q0], xT[128 * j:128 * (j + 1), q0])
        nc.scalar.dma_start(tri_sb[:], tri_d[:])
        for j in range(8):
            nc.sync.dma_start(xts[j][:, q1], xT[128 * j:128 * (j + 1), q1])
        nc.scalar.dma_start(wk_sb.rearrange("p (a c) -> p a c", c=256),
                            wk_src)
        nc.scalar.dma_start(cc_sb[:, q1], cc_d[:, q1])
        nc.scalar.dma_start(ss_sb[:, q1], ss_d[:, q1])
        for j in range(4):
            nc.sync.dma_start(xts[j][:, h1], xT[128 * j:128 * (j + 1), h1])
        nc.scalar.dma_start(cc_sb[:, h1], cc_d[:, h1])
        nc.scalar.dma_start(ss_sb[:, h1], ss_d[:, h1])
        for j in range(4, 8):
            nc.sync.dma_start(xts[j][:, h1], xT[128 * j:128 * (j + 1), h1])
        nc.scalar.dma_start(wp_sb[0][:], wp_src[:, 0, :])
        nc.scalar.dma_start(wp_sb[1][:], wp_src[:, 1, :])

        # persistent SBUF pools
        rp = ctx.enter_context(tc.tile_pool(name="rope", bufs=3))
        epl = ctx.enter_context(tc.tile_pool(name="epool", bufs=6))
        zrp = ctx.enter_context(tc.tile_pool(name="zrpool", bufs=4))
        rzbp = ctx.enter_context(tc.tile_pool(name="rzbpool", bufs=4))
        ost = ctx.enter_context(tc.tile_pool(name="ostage", bufs=6))
        pb = ctx.enter_context(tc.tile_pool(name="pbank", bufs=1,
                                            space="PSUM"))

        # PSUM: pA/pB two-bank score tiles, y0..y3 single-bank accumulators
        def pair_bank(tag, name):
            return pb.tile([128, 1024], F32, tag=tag, name=name)

        def ybank(k, name):
            return pb.tile([128, 512], F32, tag=f"y{k}", name=name)

        # filler psum: rotate over the y-banks listed in fb_state
        fb_state = {"banks": (0, 1, 2, 3), "ctr": 0}

        def fslot(name):
            banks = fb_state["banks"]
            k = banks[fb_state["ctr"] % len(banks)]
            fb_state["ctr"] += 1
            return ybank(k, name)

        # ---------------- unit definitions ----------------
        def qk_unit(isk, ci, half, tg):
            """One 512-col projection+rope unit for Q/K chunk ci."""
            csl = slice(1024 * half + 512 * tg, 1024 * half + 512 * tg + 512)
            wsl = slice(512 * tg, 512 * tg + 512)
            wsb = wk_sb if isk else wq_sb
            ps = fslot(f"qk{int(isk)}_{ci}_{half}_{tg}")
            for j in range(8):
                nc.tensor.matmul(
                    ps[:], wsb[:, 256 * j + 128 * ci:256 * j + 128 * ci + 128],
                    xts[j][:, csl], start=(j == 0), stop=(j == 7))
            bcol = (2 if isk else 0) + ci
            pair = (2 if isk else 0) + ci
            bias = bqk_sb[:, bcol:bcol + 1]
            pb16 = rp.tile([128, 512], F16, tag="pb16")
            t1 = rp.tile([128, 512], F16, tag="t1")
            shf = rp.tile([128, 512], F16, tag="shf")
            t2 = rp.tile([128, 512], F16, tag="t2")
            # DVE keeps only the shuffle; the elementwise rope ops go to the
            # otherwise-idle gpsimd so tri-adds/exp chains aren't queued
            # behind them on DVE.
            nc.scalar.activation(pb16[:], ps[:], AF.Identity, bias=bias)
            nc.vector.stream_shuffle(shf[:], pb16[:], SWAP_MASK)
            nc.gpsimd.tensor_mul(t1[:], pb16[:], cc_sb[:, csl])
            nc.gpsimd.tensor_mul(t2[:], shf[:], ss_sb[:, csl])
            nc.gpsimd.tensor_add(qkt[pair][half][:, wsl], t1[:], t2[:])

        def v_unit(i):
            """V' s-tile i: vraw = x_i^T @ Wv (+bias), copy into vp[i]."""
            ps = fslot(f"v{i}")
            tsl = slice(128 * i, 128 * (i + 1))
            for j in range(8):
                nc.tensor.matmul(ps[:, 0:256], xts[j][:, tsl],
                                 wv_sb[:, 256 * j:256 * (j + 1)],
                                 start=(j == 0), stop=False)
            nc.tensor.matmul(ps[:, 0:256], x1[:, tsl],
                             wv_sb[0:1, 2048:2304], start=False, stop=True)
            nc.scalar.activation(vview[i][:, :, 0:64], ps[:, 0:256], AF.Copy)

        yz_live = {}

        def score_round(pr, w, i, tag, pool):
            """Merged score round: both heads of pair pr, s-tile i, window w.
            Both heads land in the two banks of one [128,1024] tile; one
            merged exp (or two trimmed ones on deep-diagonal rounds)."""
            sub0 = max(0, 128 * i - 512 * w)
            kt = qkt[2 + pr][i // 8]
            qt = qkt[pr][w // 2]
            qsl = slice((512 * w) % 1024 + sub0, (512 * w) % 1024 + 512)
            ps = pair_bank(tag, f"s{pr}_{w}_{i}")
            for hs in range(2):
                rows = slice(64 * hs, 64 * (hs + 1))
                nc.tensor.matmul(
                    ps[:, 512 * hs + sub0:512 * (hs + 1)],
                    kt[rows, 128 * (i % 8):128 * (i % 8) + 128],
                    qt[rows, qsl], start=True, stop=True)
            if i >= 4 * w:
                for hs in range(2):
                    o = 512 * hs + sub0
                    nc.vector.tensor_add(ps[:, o:o + 128], ps[:, o:o + 128],
                                         tri_sb[:])
            et = pool.tile([128, 1024], F16, tag="e", name=f"e{pr}_{w}_{i}")
            if sub0 > 0:
                for hs in range(2):
                    o = 512 * hs + sub0
                    e = 512 * (hs + 1)
                    nc.scalar.activation(et[:, o:e], ps[:, o:e], AF.Exp,
                                         scale=0.125)
            else:
                nc.scalar.activation(et[:], ps[:], AF.Exp, scale=0.125)
            return et, sub0

        def attv_round(pr, w, i, et, sub0, ni):
            for hs in range(2):
                h = 2 * pr + hs
                if i == 0:
                    yz_live[h] = ybank(h, f"yz{h}_{w}")
                nc.tensor.matmul(
                    yz_live[h][0:65, sub0:512],
                    vp[i][:, 65 * h:65 * (h + 1)],
                    et[:, 512 * hs + sub0:512 * (hs + 1)],
                    start=(i == 0), stop=(i == ni - 1))

        def norm_pair(pr, w):
            """Normalize both heads of pair pr for window w into usc."""
            rz, rb = {}, {}
            zr = {}
            for hs in range(2):
                h = 2 * pr + hs
                # PSUM holds e10m23; the recip's bitwise seed needs IEEE fp32
                # bits, so bounce z through SBUF via the (idle) ACT engine.
                zr[hs] = zrp.tile([1, 512], F32, tag="zrow", name=f"zc{h}_{w}")
                nc.scalar.activation(zr[hs][:], yz_live[h][64:65, :], AF.Copy)
            for hs in range(2):
                h = 2 * pr + hs
                rz[hs] = zrp.tile([1, 512], F32, tag="rzr", name=f"rr{h}_{w}")
                nc.vector.reciprocal_approx_fast(rz[hs][:], zr[hs][:])
            for hs in range(2):
                h = 2 * pr + hs
                rb[hs] = rzbp.tile([64, 512], F32, tag="rzb",
                                   name=f"rb{h}_{w}")
                nc.gpsimd.partition_broadcast(rb[hs][:], rz[hs][:])
            for hs in range(2):
                h = 2 * pr + hs
                nc.vector.tensor_mul(usc[pr][w][64 * hs:64 * (hs + 1), :],
                                     yz_live[h][0:64, :], rb[hs][:])

        def proj_unit(tch, cg, psl, on_act=False):
            """Output projection for t-chunk tch, 512-col group cg."""
            w = tch // 4
            tsl = slice(128 * (tch % 4), 128 * (tch % 4) + 128)
            csl = slice(512 * cg, 512 * (cg + 1))
            for pq in range(2):
                nc.tensor.matmul(psl[:, 0:512], usc[pq][w][:, tsl],
                                 wp_sb[pq][:, csl],
                                 start=(pq == 0), stop=(pq == 1))
            st = ost.tile([128, 512], F16, tag="ost", name=f"st{tch}_{cg}")
            if on_act:
                nc.scalar.activation(st[:], psl[:, 0:512], AF.Copy)
            else:
                nc.vector.tensor_copy(st[:], psl[:, 0:512])
            nc.sync.dma_start(out_d[128 * tch:128 * tch + 128, csl], st[:])

        # ---------------- A: startup stream ----------------
        # v0-3 rotate y0..y3; the first two qk units go once more through
        # y0/y1 (their readers are long done before w0's yz claims them).
        fb_state["banks"] = (0, 1, 2, 3)
        for i in range(4):
            v_unit(i)
        qk_unit(False, 0, 0, 0)
        qk_unit(True, 0, 0, 0)

        # ---------------- B: w0/w1 per pair + filler ----------------
        def window(w, pr, fill, nfill_share):
            """Attention window w for pair pr; scores alternate pA/pB,
            attV lags LAG rounds; fillers popped between."""
            ni = 4 * w + 4
            nr = ni + LAG
            pend = deque()
            emitted = 0
            for r in range(nr):
                if r < ni:
                    et, sub0 = score_round(pr, w, r,
                                           "pA" if r % 2 == 0 else "pB", epl)
                    pend.append((r, et, sub0))
                while fill and emitted < (r + 1) * nfill_share // nr:
                    fill.popleft()()
                    emitted += 1
                if r >= LAG:
                    i, et, sub0 = pend.popleft()
                    attv_round(pr, w, i, et, sub0, ni)
            norm_pair(pr, w)

        fill = deque()
        fill.append(lambda: qk_unit(False, 1, 0, 0))
        fill.append(lambda: qk_unit(True, 1, 0, 0))
        for isk in (False, True):
            fill.append(lambda isk=isk: qk_unit(isk, 0, 0, 1))
        for i in range(4, 8):
            fill.append(lambda i=i: v_unit(i))
        for isk in (False, True):
            fill.append(lambda isk=isk: qk_unit(isk, 1, 0, 1))
        for isk in (False, True):
            for tg in range(2):
                fill.append(lambda isk=isk, tg=tg: qk_unit(isk, 0, 1, tg))
        for i in range(8, 12):
            fill.append(lambda i=i: v_unit(i))
        for isk in (False, True):
            for tg in range(2):
                fill.append(lambda isk=isk, tg=tg: qk_unit(isk, 1, 1, tg))
        for i in range(12, 16):
            fill.append(lambda i=i: v_unit(i))

        nf = len(fill)
        shares = [nf * 5 // 32, nf * 12 // 32, nf * 22 // 32, nf]
        prev = 0
        for wi, (w, pr) in enumerate([(0, 0), (0, 1), (1, 0), (1, 1)]):
            # fillers use the y-banks of the inactive pair
            fb_state["banks"] = (2, 3) if pr == 0 else (0, 1)
            cnt = shares[wi] - prev
            prev = shares[wi]
            sub = deque(fill.popleft() for _ in range(cnt))
            window(w, pr, sub, cnt)
            while sub:
                sub.popleft()()

        # ---------------- C: w2/w3 as per-pair windows + proj filler -------
        # proj t-chunks become available as the usc windows complete:
        # tch0-7 after B, tch8-11 after both w2 norms, tch12-15 at the end.
        def projf(tch, cg, on_act=False):
            return lambda: proj_unit(tch, cg, fslot(f"op{tch}_{cg}"),
                                     on_act=on_act)

        fill = deque(projf(tch, cg) for tch in range(4) for cg in range(2))
        fb_state["banks"] = (2, 3)
        window(2, 0, fill, len(fill))
        fill = deque(projf(tch, cg) for tch in range(4, 8) for cg in range(2))
        fb_state["banks"] = (0, 1)
        window(2, 1, fill, len(fill))

        fill = deque(projf(tch, cg, True) for tch in range(8, 10)
                     for cg in range(2))
        fb_state["banks"] = (2, 3)
        window(3, 0, fill, len(fill))
        fill = deque(projf(tch, cg, True) for tch in range(10, 12)
                     for cg in range(2))
        fb_state["banks"] = (0, 1)
        window(3, 1, fill, len(fill))

        # ---------------- E: tail projections on pA/pB halves -------------
        pr_slots = {"ctr": 0, "cur": {}}

        def pslot(name):
            k = pr_slots["ctr"] % 4
            pr_slots["ctr"] += 1
            tag = "pA" if k < 2 else "pB"
            half = k % 2
            if half == 0:
                pr_slots["cur"][tag] = pair_bank(tag, name)
            return pr_slots["cur"][tag][:, 512 * half:512 * (half + 1)]

        for tch in range(12, 16):
            for cg in range(2):
                proj_unit(tch, cg, pslot(f"op{tch}_{cg}"), on_act=True)

    nc.compile()
    return nc


_NC_CACHE = {}


def _get_nc():
    if "nc" not in _NC_CACHE:
        _NC_CACHE["nc"] = build_nc()
    return _NC_CACHE["nc"]


def make_in_map(core, x, Wq, bq, Wk, bk, Wv, bv, Wp, bp, rope_cache):
    b = core // 4
    hbase = (core % 4) * 4

    xTa = np.empty((C + 1, T), np.float16)
    xTa[:C] = np.asarray(x[b], np.float32).T
    xTa[C] = 1.0

    # packed channel order for Q/K: per head, two 32-row quadrants; each
    # quadrant holds [even ch 16q..16q+15 | odd ch 16q..16q+15] so the rope
    # partner swap is lane l -> (l+16)%32 inside every quadrant.
    perm = []
    for p in range(2):
        for hh in range(2):
            h = hbase + 2 * p + hh
            for q in range(2):
                perm += [h * HD + 2 * (16 * q + m) for m in range(16)]
                perm += [h * HD + 2 * (16 * q + m) + 1 for m in range(16)]
    perm = np.asarray(perm)

    wqTa = np.ascontiguousarray(
        np.asarray(Wq, np.float32)[perm, :].T).astype(np.float16)
    wkTa = np.ascontiguousarray(
        np.asarray(Wk, np.float32)[perm, :].T).astype(np.float16)

    chs = np.arange(hbase * HD, hbase * HD + 256)
    wvTa = np.zeros((C + 128, 256), np.float16)
    wvTa[:C] = np.asarray(Wv, np.float32)[chs, :].T
    wvTa[C] = np.asarray(bv, np.float32)[chs]
    wpTa = np.ascontiguousarray(
        np.asarray(Wp, np.float32)[:, chs].T).astype(np.float16)

    bqp = np.asarray(bq, np.float32)[perm].reshape(2, 128).T
    bkp = np.asarray(bk, np.float32)[perm].reshape(2, 128).T
    bqk_a = np.concatenate([bqp, bkp], axis=1)  # [128, 4]

    rc = np.asarray(rope_cache, np.float32)  # [T, 32, 2]
    r = np.arange(128)
    lane = r % 32
    quad = (r // 32) % 2
    m = 16 * quad + (lane % 16)  # rotation pair index per row
    sign = np.where(lane < 16, 1.0, -1.0).astype(np.float32)
    cc_a = np.ascontiguousarray(rc[:, m, 0].T).astype(np.float16)
    ss_raw = (rc[:, m, 1].T * sign[:, None]).astype(np.float16)
    # pre-shuffle sin rows so t2 = shuffle(pb) * ss_pre == shuffle(pb * ss)
    swap = np.asarray(SWAP_MASK)
    rows = np.arange(128)
    src = (rows // 32) * 32 + swap[rows % 32]
    ss_a = np.ascontiguousarray(ss_raw[src, :])

    sl, tl = np.arange(128)[:, None], np.arange(128)[None, :]
    tri_a = np.where(tl >= sl, 0.0, NEG).astype(np.float32)

    return dict(xT=xTa, wqT=wqTa, wkT=wkTa, wvT=wvTa, wpT=wpTa,
                bqk=bqk_a, cc=cc_a, ss=ss_a, tri=tri_a)


def kernel(x, Wq, bq, Wk, bk, Wv, bv, Wp, bp, rope_cache):
    global LAST_EXEC_NS, LAST_RESULTS
    args = (x, Wq, bq, Wk, bk, Wv, bv, Wp, bp, rope_cache)
    nc = _get_nc()
    in_maps = [make_in_map(c, *args) for c in range(NCORES)]
    r = None
    for attempt in range(4):
        try:
            r = run_bass_kernel_spmd(nc, in_maps, list(range(NCORES)))
            break
        except Exception:
            # transient NRT exec-unit errors recover on re-dispatch
            if attempt == 3:
                raise
            time.sleep(5.0 * (attempt + 1))
    LAST_EXEC_NS = r.exec_time_ns
    LAST_RESULTS = r
    out = np.zeros((2, T, C), np.float32)
    for core in range(NCORES):
        out[core // 4] += np.asarray(r.results[core]["out"], np.float32)
    out += np.asarray(bp, np.float32)[None, None, :]
    return out


# revision 20
# speedup vs baseline: 1.0758x; 1.0758x over previous
"""Trainium2 Bass kernel for causal self-attention with RoPE.

Problem: B=2, T=2048, C=1024, H=16 heads, hd=64, fp32 in/out, causal, rotary.

Sharding: 8 cores = 2 batches x 4 head-groups. Core c handles batch c//4 and
heads [4*(c%4), 4*(c%4)+4). Each core computes its heads' Q/K/V projections,
RoPE, causal attention, and a partial output projection over its 256 input
channels; the host sums the 4 partial projections per batch and adds bp.

v2 design notes (from the v1 trace: ACT/exp is the co-bottleneck with PE,
and the v1 tail serialized on norm chains):
 - The two heads of a pair write adjacent PSUM banks of one [128,1024] tile
   and a single EXP covers both (1147ns vs 2x720ns); deep-diagonal rounds
   use two trimmed exps instead.
 - PSUM tags: pA/pB = two [128,1024] score tiles (2 banks each), y0..y3 =
   per-head attV accumulators (64 y-rows + z-row). Attention windows run
   per-pair so the inactive pair's y-banks serve as filler/proj PSUM.
 - Rope: ACT does the bias-add + fp32->fp16 cast out of PSUM (Identity with
   per-partition bias); sin is pre-shuffled on the host so DVE does only
   mul/shuffle/mul/add on fp16.
 - All output projections run as PE filler inside the ACT-bound score phases
   or the attV-w3 phase; out-DMA goes per 512-col chunk after each cast.
 - Input DMA: few big multi-dim transfers (v1 startup was sync-issue bound),
   ordered so V-units (cheapest deps) start first; issued from 4 engines.
 - Norm chains: reciprocal reads z directly from PSUM, gpsimd broadcasts,
   DVE multiplies; per-head chains are emitted stage-interleaved.
"""

import time
from collections import deque
from contextlib import ExitStack

import numpy as np

import concourse.bass as bass
import concourse.tile as tile
from concourse import bacc, library_config, mybir
from concourse.bass_utils import run_bass_kernel_spmd

F32 = mybir.dt.float32
F16 = mybir.dt.float16

T = 2048
C = 1024
HD = 64
NCORES = 8
NEG = -1e10
LAG = 3
SWAP_MASK = list(range(16, 32)) + list(range(16))

AF = mybir.ActivationFunctionType
ALU = mybir.AluOpType

LAST_EXEC_NS = None
LAST_RESULTS = None


def build_nc():
    nc = bacc.Bacc("TRN2", target_bir_lowering=False, debug=False)

    xT = nc.dram_tensor("xT", [C + 1, T], F16, kind="ExternalInput").ap()
    wqT = nc.dram_tensor("wqT", [C, 256], F16, kind="ExternalInput").ap()
    wkT = nc.dram_tensor("wkT", [C, 256], F16, kind="ExternalInput").ap()
    wvT = nc.dram_tensor("wvT", [C + 128, 256], F16, kind="ExternalInput").ap()
    wpT = nc.dram_tensor("wpT", [256, C], F16, kind="ExternalInput").ap()
    bqk = nc.dram_tensor("bqk", [128, 4], F32, kind="ExternalInput").ap()
    cc_d = nc.dram_tensor("cc", [128, T], F16, kind="ExternalInput").ap()
    ss_d = nc.dram_tensor("ss", [128, T], F16, kind="ExternalInput").ap()
    tri_d = nc.dram_tensor("tri", [128, 128], F32, kind="ExternalInput").ap()
    out_d = nc.dram_tensor("out", [T, C], F16, kind="ExternalOutput").ap()

    with tile.TileContext(nc) as tc, ExitStack() as ctx:
        consts = ctx.enter_context(tc.tile_pool(name="consts", bufs=1))

        cc_sb = consts.tile([128, T], F16)
        ss_sb = consts.tile([128, T], F16)   # pre-shuffled+signed sin
        tri_sb = consts.tile([128, 128], F32)
        bqk_sb = consts.tile([128, 4], F32)
        x1 = consts.tile([1, T], F16)

        # rotated Q^T / K^T: [pair][half] tiles (Q pairs 0-1, K pairs 2-3)
        qkt = [[consts.tile([128, 1024], F16, name=f"qkt{p}_{h}")
                for h in range(2)] for p in range(4)]
        vp = [consts.tile([128, 4 * 65], F16, name=f"vp{i}") for i in range(16)]
        vview = [v.rearrange("p (h d) -> p h d", d=65) for v in vp]
        usc = [[consts.tile([128, 512], F16, name=f"usc{p}_{w}")
                for w in range(4)] for p in range(2)]
        wp_sb = [consts.tile([128, C], F16, name=f"wp{p}") for p in range(2)]
        xts = [consts.tile([128, T], F16, name=f"xt{j}") for j in range(8)]
        # packed weights: 8 (9 for V) row-chunks side by side in the free dim
        wq_sb = consts.tile([128, 8 * 256], F16)
        wk_sb = consts.tile([128, 8 * 256], F16)
        wv_sb = consts.tile([128, 9 * 256], F16)

        h0, h1 = slice(0, 1024), slice(1024, 2048)

        # ---------- input DMA: few big transfers, priority order ----------
        wv_src = wvT.rearrange("(a p) c -> p a c", p=128)
        wq_src = wqT.rearrange("(a p) c -> p a c", p=128)
        wk_src = wkT.rearrange("(a p) c -> p a c", p=128)
        wp_src = wpT.rearrange("(a p) c -> p a c", p=128)

        # gpsimd: memsets first (they gate the V units), then the library;
        # no DMAs here — SWDGE issue costs ~1.7us each.
        nc.gpsimd.memset(x1[:], 1.0)
        for i in range(16):
            nc.gpsimd.memset(vview[i][:, :, 64], 1.0)
        nc.gpsimd.load_library(library_config.attn)

        q0, q1 = slice(0, 512), slice(512, 1024)
        nc.sync.dma_start(wv_sb.rearrange("p (a c) -> p a c", c=256), wv_src)
        nc.scalar.dma_start(bqk_sb[:], bqk[:])
        nc.scalar.dma_start(cc_sb[:, q0], cc_d[:, q0])
        nc.scalar.dma_start(ss_sb[:, q0], ss_d[:, q0])
        nc.scalar.dma_start(wq_sb.rearrange("p (a c) -> p a c", c=256),
                            wq_src)
        for j in range(8):
            nc.sync.dma_start(xts[j][:, q0], xT[128 * j:128 * (j + 1), q0])
        nc.scalar.dma_start(tri_sb[:], tri_d[:])
        for j in range(8):
            nc.sync.dma_start(xts[j][:, q1], xT[128 * j:128 * (j + 1), q1])
        nc.scalar.dma_start(wk_sb.rearrange("p (a c) -> p a c", c=256),
                            wk_src)
        nc.scalar.dma_start(cc_sb[:, q1], cc_d[:, q1])
        nc.scalar.dma_start(ss_sb[:, q1], ss_d[:, q1])
        for j in range(4):
            nc.sync.dma_start(xts[j][:, h1], xT[128 * j:128 * (j + 1), h1])
        nc.scalar.dma_start(cc_sb[:, h1], cc_d[:, h1])
        nc.scalar.dma_start(ss_sb[:, h1], ss_d[:, h1])
        for j in range(4, 8):
            nc.sync.dma_start(xts[j][:, h1], xT[128 * j:128 * (j + 1), h1])
        nc.scalar.dma_start(wp_sb[0][:], wp_src[:, 0, :])
        nc.scalar.dma_start(wp_sb[1][:], wp_src[:, 1, :])

        # persistent SBUF pools
        rp = ctx.enter_context(tc.tile_pool(name="rope", bufs=3))
        epl = ctx.enter_context(tc.tile_pool(name="epool", bufs=6))
        zrp = ctx.enter_context(tc.tile_pool(name="zrpool", bufs=4))
        rzbp = ctx.enter_context(tc.tile_pool(name="rzbpool", bufs=4))
        ost = ctx.enter_context(tc.tile_pool(name="ostage", bufs=6))
        pb = ctx.enter_context(tc.tile_pool(name="pbank", bufs=1,
                                            space="PSUM"))

        # PSUM: pA/pB two-bank score tiles, y0..y3 single-bank accumulators
        def pair_bank(tag, name):
            return pb.tile([128, 1024], F32, tag=tag, name=name)

        def ybank(k, name):
            return pb.tile([128, 512], F32, tag=f"y{k}", name=name)

        # filler psum: rotate over the y-banks listed in fb_state
        fb_state = {"banks": (0, 1, 2, 3), "ctr": 0}

        def fslot(name):
            banks = fb_state["banks"]
            k = banks[fb_state["ctr"] % len(banks)]
            fb_state["ctr"] += 1
            return ybank(k, name)

        # ---------------- unit definitions ----------------
        rope_ctr = [0]

        def qk_unit(isk, ci, half, tg):
            """One 512-col projection+rope unit for Q/K chunk ci."""
            csl = slice(1024 * half + 512 * tg, 1024 * half + 512 * tg + 512)
            wsl = slice(512 * tg, 512 * tg + 512)
            wsb = wk_sb if isk else wq_sb
            ps = fslot(f"qk{int(isk)}_{ci}_{half}_{tg}")
            for j in range(8):
                nc.tensor.matmul(
                    ps[:], wsb[:, 256 * j + 128 * ci:256 * j + 128 * ci + 128],
                    xts[j][:, csl], start=(j == 0), stop=(j == 7))
            bcol = (2 if isk else 0) + ci
            pair = (2 if isk else 0) + ci
            bias = bqk_sb[:, bcol:bcol + 1]
            pb16 = rp.tile([128, 512], F16, tag="pb16")
            t1 = rp.tile([128, 512], F16, tag="t1")
            shf = rp.tile([128, 512], F16, tag="shf")
            t2 = rp.tile([128, 512], F16, tag="t2")
            # Alternate rope units between DVE and gpsimd so window tri-adds
            # aren't queued behind every rope chain on DVE.
            nc.scalar.activation(pb16[:], ps[:], AF.Identity, bias=bias)
            nc.vector.stream_shuffle(shf[:], pb16[:], SWAP_MASK)
            eng = nc.gpsimd if (rope_ctr[0] % 2 == 0) else nc.vector
            rope_ctr[0] += 1
            eng.tensor_mul(t1[:], pb16[:], cc_sb[:, csl])
            eng.tensor_mul(t2[:], shf[:], ss_sb[:, csl])
            eng.tensor_add(qkt[pair][half][:, wsl], t1[:], t2[:])

        def v_unit(i):
            """V' s-tile i: vraw = x_i^T @ Wv (+bias), copy into vp[i]."""
            ps = fslot(f"v{i}")
            tsl = slice(128 * i, 128 * (i + 1))
            for j in range(8):
                nc.tensor.matmul(ps[:, 0:256], xts[j][:, tsl],
                                 wv_sb[:, 256 * j:256 * (j + 1)],
                                 start=(j == 0), stop=False)
            nc.tensor.matmul(ps[:, 0:256], x1[:, tsl],
                             wv_sb[0:1, 2048:2304], start=False, stop=True)
            nc.scalar.activation(vview[i][:, :, 0:64], ps[:, 0:256], AF.Copy)

        yz_live = {}

        def score_round(pr, w, i, tag, pool):
            """Merged score round: both heads of pair pr, s-tile i, window w.
            Both heads land in the two banks of one [128,1024] tile; one
            merged exp (or two trimmed ones on deep-diagonal rounds)."""
            sub0 = max(0, 128 * i - 512 * w)
            kt = qkt[2 + pr][i // 8]
            qt = qkt[pr][w // 2]
            qsl = slice((512 * w) % 1024 + sub0, (512 * w) % 1024 + 512)
            ps = pair_bank(tag, f"s{pr}_{w}_{i}")
            for hs in range(2):
                rows = slice(64 * hs, 64 * (hs + 1))
                nc.tensor.matmul(
                    ps[:, 512 * hs + sub0:512 * (hs + 1)],
                    kt[rows, 128 * (i % 8):128 * (i % 8) + 128],
                    qt[rows, qsl], start=True, stop=True)
            if i >= 4 * w:
                for hs in range(2):
                    o = 512 * hs + sub0
                    nc.vector.tensor_add(ps[:, o:o + 128], ps[:, o:o + 128],
                                         tri_sb[:])
            et = pool.tile([128, 1024], F16, tag="e", name=f"e{pr}_{w}_{i}")
            if sub0 > 0:
                for hs in range(2):
                    o = 512 * hs + sub0
                    e = 512 * (hs + 1)
                    nc.scalar.activation(et[:, o:e], ps[:, o:e], AF.Exp,
                                         scale=0.125)
            else:
                nc.scalar.activation(et[:], ps[:], AF.Exp, scale=0.125)
            return et, sub0

        def attv_round(pr, w, i, et, sub0, ni):
            for hs in range(2):
                h = 2 * pr + hs
                if i == 0:
                    yz_live[h] = ybank(h, f"yz{h}_{w}")
                nc.tensor.matmul(
                    yz_live[h][0:65, sub0:512],
                    vp[i][:, 65 * h:65 * (h + 1)],
                    et[:, 512 * hs + sub0:512 * (hs + 1)],
                    start=(i == 0), stop=(i == ni - 1))

        def norm_pair(pr, w):
            """Normalize both heads of pair pr for window w into usc."""
            rz, rb = {}, {}
            zr = {}
            for hs in range(2):
                h = 2 * pr + hs
                # PSUM holds e10m23; the recip's bitwise seed needs IEEE fp32
                # bits, so bounce z through SBUF via the (idle) ACT engine.
                zr[hs] = zrp.tile([1, 512], F32, tag="zrow", name=f"zc{h}_{w}")
                nc.scalar.activation(zr[hs][:], yz_live[h][64:65, :], AF.Copy)
            for hs in range(2):
                h = 2 * pr + hs
                rz[hs] = zrp.tile([1, 512], F32, tag="rzr", name=f"rr{h}_{w}")
                nc.vector.reciprocal_approx_fast(rz[hs][:], zr[hs][:])
            for hs in range(2):
                h = 2 * pr + hs
                rb[hs] = rzbp.tile([64, 512], F32, tag="rzb",
                                   name=f"rb{h}_{w}")
                nc.gpsimd.partition_broadcast(rb[hs][:], rz[hs][:])
            for hs in range(2):
                h = 2 * pr + hs
                nc.vector.tensor_mul(usc[pr][w][64 * hs:64 * (hs + 1), :],
                                     yz_live[h][0:64, :], rb[hs][:])

        def proj_unit(tch, cg, psl, on_act=False):
            """Output projection for t-chunk tch, 512-col group cg."""
            w = tch // 4
            tsl = slice(128 * (tch % 4), 128 * (tch % 4) + 128)
            csl = slice(512 * cg, 512 * (cg + 1))
            for pq in range(2):
                nc.tensor.matmul(psl[:, 0:512], usc[pq][w][:, tsl],
                                 wp_sb[pq][:, csl],
                                 start=(pq == 0), stop=(pq == 1))
            st = ost.tile([128, 512], F16, tag="ost", name=f"st{tch}_{cg}")
            if on_act:
                nc.scalar.activation(st[:], psl[:, 0:512], AF.Copy)
            else:
                nc.vector.tensor_copy(st[:], psl[:, 0:512])
            nc.sync.dma_start(out_d[128 * tch:128 * tch + 128, csl], st[:])

        # ---------------- A: startup stream ----------------
        # v0-3 rotate y0..y3; the first two qk units go once more through
        # y0/y1 (their readers are long done before w0's yz claims them).
        fb_state["banks"] = (0, 1, 2, 3)
        for i in range(4):
            v_unit(i)
        qk_unit(False, 0, 0, 0)
        qk_unit(True, 0, 0, 0)

        # ---------------- B: w0/w1 per pair + filler ----------------
        def window(w, pr, fill, nfill_share):
            """Attention window w for pair pr; scores alternate pA/pB,
            attV lags LAG rounds; fillers popped between."""
            ni = 4 * w + 4
            nr = ni + LAG
            pend = deque()
            emitted = 0
            for r in range(nr):
                if r < ni:
                    et, sub0 = score_round(pr, w, r,
                                           "pA" if r % 2 == 0 else "pB", epl)
                    pend.append((r, et, sub0))
                while fill and emitted < (r + 1) * nfill_share // nr:
                    fill.popleft()()
                    emitted += 1
                if r >= LAG:
                    i, et, sub0 = pend.popleft()
                    attv_round(pr, w, i, et, sub0, ni)
            norm_pair(pr, w)

        fill = deque()
        fill.append(lambda: qk_unit(False, 1, 0, 0))
        fill.append(lambda: qk_unit(True, 1, 0, 0))
        for isk in (False, True):
            fill.append(lambda isk=isk: qk_unit(isk, 0, 0, 1))
        for i in range(4, 8):
            fill.append(lambda i=i: v_unit(i))
        for isk in (False, True):
            fill.append(lambda isk=isk: qk_unit(isk, 1, 0, 1))
        for isk in (False, True):
            for tg in range(2):
                fill.append(lambda isk=isk, tg=tg: qk_unit(isk, 0, 1, tg))
        for i in range(8, 12):
            fill.append(lambda i=i: v_unit(i))
        for isk in (False, True):
            for tg in range(2):
                fill.append(lambda isk=isk, tg=tg: qk_unit(isk, 1, 1, tg))
        for i in range(12, 16):
            fill.append(lambda i=i: v_unit(i))

        nf = len(fill)
        shares = [nf * 5 // 32, nf * 12 // 32, nf * 22 // 32, nf]
        prev = 0
        for wi, (w, pr) in enumerate([(0, 0), (0, 1), (1, 0), (1, 1)]):
            # fillers use the y-banks of the inactive pair
            fb_state["banks"] = (2, 3) if pr == 0 else (0, 1)
            cnt = shares[wi] - prev
            prev = shares[wi]
            sub = deque(fill.popleft() for _ in range(cnt))
            window(w, pr, sub, cnt)
            while sub:
                sub.popleft()()

        # ---------------- C: w2/w3 as per-pair windows + proj filler -------
        # proj t-chunks become available as the usc windows complete:
        # tch0-7 after B, tch8-11 after both w2 norms, tch12-15 at the end.
        def projf(tch, cg, on_act=False):
            return lambda: proj_unit(tch, cg, fslot(f"op{tch}_{cg}"),
                                     on_act=on_act)

        fill = deque(projf(tch, cg) for tch in range(4) for cg in range(2))
        fb_state["banks"] = (2, 3)
        window(2, 0, fill, len(fill))
        fill = deque(projf(tch, cg) for tch in range(4, 8) for cg in range(2))
        fb_state["banks"] = (0, 1)
        window(2, 1, fill, len(fill))

        fill = deque(projf(tch, cg, True) for tch in range(8, 10)
                     for cg in range(2))
        fb_state["banks"] = (2, 3)
        window(3, 0, fill, len(fill))
        fill = deque(projf(tch, cg, True) for tch in range(10, 12)
                     for cg in range(2))
        fb_state["banks"] = (0, 1)
        window(3, 1, fill, len(fill))

        # ---------------- E: tail projections on pA/pB halves -------------
        pr_slots = {"ctr": 0, "cur": {}}

        def pslot(name):
            k = pr_slots["ctr"] % 4
            pr_slots["ctr"] += 1
            tag = "pA" if k < 2 else "pB"
            half = k % 2
            if half == 0:
                pr_slots["cur"][tag] = pair_bank(tag, name)
            return pr_slots["cur"][tag][:, 512 * half:512 * (half + 1)]

        for tch in range(12, 16):
            for cg in range(2):
                proj_unit(tch, cg, pslot(f"op{tch}_{cg}"), on_act=True)

    nc.compile()
    return nc


_NC_CACHE = {}


def _get_nc():
    if "nc" not in _NC_CACHE:
        _NC_CACHE["nc"] = build_nc()
    return _NC_CACHE["nc"]


def make_in_map(core, x, Wq, bq, Wk, bk, Wv, bv, Wp, bp, rope_cache):
    b = core // 4
    hbase = (core % 4) * 4

    xTa = np.empty((C + 1, T), np.float16)
    xTa[:C] = np.asarray(x[b], np.float32).T
    xTa[C] = 1.0

    # packed channel order for Q/K: per head, two 32-row quadrants; each
    # quadrant holds [even ch 16q..16q+15 | odd ch 16q..16q+15] so the rope
    # partner swap is lane l -> (l+16)%32 inside every quadrant.
    perm = []
    for p in range(2):
        for hh in range(2):
            h = hbase + 2 * p + hh
            for q in range(2):
                perm += [h * HD + 2 * (16 * q + m) for m in range(16)]
                perm += [h * HD + 2 * (16 * q + m) + 1 for m in range(16)]
    perm = np.asarray(perm)

    wqTa = np.ascontiguousarray(
        np.asarray(Wq, np.float32)[perm, :].T).astype(np.float16)
    wkTa = np.ascontiguousarray(
        np.asarray(Wk, np.float32)[perm, :].T).astype(np.float16)

    chs = np.arange(hbase * HD, hbase * HD + 256)
    wvTa = np.zeros((C + 128, 256), np.float16)
    wvTa[:C] = np.asarray(Wv, np.float32)[chs, :].T
    wvTa[C] = np.asarray(bv, np.float32)[chs]
    wpTa = np.ascontiguousarray(
        np.asarray(Wp, np.float32)[:, chs].T).astype(np.float16)

    bqp = np.asarray(bq, np.float32)[perm].reshape(2, 128).T
    bkp = np.asarray(bk, np.float32)[perm].reshape(2, 128).T
    bqk_a = np.concatenate([bqp, bkp], axis=1)  # [128, 4]

    rc = np.asarray(rope_cache, np.float32)  # [T, 32, 2]
    r = np.arange(128)
    lane = r % 32
    quad = (r // 32) % 2
    m = 16 * quad + (lane % 16)  # rotation pair index per row
    sign = np.where(lane < 16, 1.0, -1.0).astype(np.float32)
    cc_a = np.ascontiguousarray(rc[:, m, 0].T).astype(np.float16)
    ss_raw = (rc[:, m, 1].T * sign[:, None]).astype(np.float16)
    # pre-shuffle sin rows so t2 = shuffle(pb) * ss_pre == shuffle(pb * ss)
    swap = np.asarray(SWAP_MASK)
    rows = np.arange(128)
    src = (rows // 32) * 32 + swap[rows % 32]
    ss_a = np.ascontiguousarray(ss_raw[src, :])

    sl, tl = np.arange(128)[:, None], np.arange(128)[None, :]
    tri_a = np.where(tl >= sl, 0.0, NEG).astype(np.float32)

    return dict(xT=xTa, wqT=wqTa, wkT=wkTa, wvT=wvTa, wpT=wpTa,
                bqk=bqk_a, cc=cc_a, ss=ss_a, tri=tri_a)


def kernel(x, Wq, bq, Wk, bk, Wv, bv, Wp, bp, rope_cache):
    global LAST_EXEC_NS, LAST_RESULTS
    args = (x, Wq, bq, Wk, bk, Wv, bv, Wp, bp, rope_cache)
    nc = _get_nc()
    in_maps = [make_in_map(c, *args) for c in range(NCORES)]
    r = None
    for attempt in range(4):
        try:
            r = run_bass_kernel_spmd(nc, in_maps, list(range(NCORES)))
            break
        except Exception:
            # transient NRT exec-unit errors recover on re-dispatch
            if attempt == 3:
                raise
            time.sleep(5.0 * (attempt + 1))
    LAST_EXEC_NS = r.exec_time_ns
    LAST_RESULTS = r
    out = np.zeros((2, T, C), np.float32)
    for core in range(NCORES):
        out[core // 4] += np.asarray(r.results[core]["out"], np.float32)
    out += np.asarray(bp, np.float32)[None, None, :]
    return out


# revision 21
# speedup vs baseline: 1.1990x; 1.1145x over previous
"""Trainium2 Bass kernel for causal self-attention with RoPE.

Problem: B=2, T=2048, C=1024, H=16 heads, hd=64, fp32 in/out, causal, rotary.

Sharding: 8 cores = 2 batches x 4 head-groups. Core c handles batch c//4 and
heads [4*(c%4), 4*(c%4)+4). Each core computes its heads' Q/K/V projections,
RoPE, causal attention, and a partial output projection over its 256 input
channels; the host sums the 4 partial projections per batch and adds bp.

v2 design notes (from the v1 trace: ACT/exp is the co-bottleneck with PE,
and the v1 tail serialized on norm chains):
 - The two heads of a pair write adjacent PSUM banks of one [128,1024] tile
   and a single EXP covers both (1147ns vs 2x720ns); deep-diagonal rounds
   use two trimmed exps instead.
 - PSUM tags: pA/pB = two [128,1024] score tiles (2 banks each), y0..y3 =
   per-head attV accumulators (64 y-rows + z-row). Attention windows run
   per-pair so the inactive pair's y-banks serve as filler/proj PSUM.
 - Rope: ACT does the bias-add + fp32->fp16 cast out of PSUM (Identity with
   per-partition bias); sin is pre-shuffled on the host so DVE does only
   mul/shuffle/mul/add on fp16.
 - All output projections run as PE filler inside the ACT-bound score phases
   or the attV-w3 phase; out-DMA goes per 512-col chunk after each cast.
 - Input DMA: few big multi-dim transfers (v1 startup was sync-issue bound),
   ordered so V-units (cheapest deps) start first; issued from 4 engines.
 - Norm chains: reciprocal reads z directly from PSUM, gpsimd broadcasts,
   DVE multiplies; per-head chains are emitted stage-interleaved.
"""

import time
from collections import deque
from contextlib import ExitStack

import numpy as np

import concourse.bass as bass
import concourse.tile as tile
from concourse import bacc, library_config, mybir
from concourse.bass_utils import run_bass_kernel_spmd

F32 = mybir.dt.float32
F16 = mybir.dt.float16

T = 2048
C = 1024
HD = 64
NCORES = 8
NEG = -1e10
LAG = 3
SWAP_MASK = list(range(16, 32)) + list(range(16))

AF = mybir.ActivationFunctionType
ALU = mybir.AluOpType

LAST_EXEC_NS = None
LAST_RESULTS = None


def build_nc():
    nc = bacc.Bacc("TRN2", target_bir_lowering=False, debug=False)

    xT = nc.dram_tensor("xT", [C + 1, T], F16, kind="ExternalInput").ap()
    wqT = nc.dram_tensor("wqT", [C, 256], F16, kind="ExternalInput").ap()
    wkT = nc.dram_tensor("wkT", [C, 256], F16, kind="ExternalInput").ap()
    wvT = nc.dram_tensor("wvT", [C + 128, 256], F16, kind="ExternalInput").ap()
    wpT = nc.dram_tensor("wpT", [256, C], F16, kind="ExternalInput").ap()
    bqk = nc.dram_tensor("bqk", [128, 4], F32, kind="ExternalInput").ap()
    cc_d = nc.dram_tensor("cc", [128, T], F16, kind="ExternalInput").ap()
    ss_d = nc.dram_tensor("ss", [128, T], F16, kind="ExternalInput").ap()
    tri_d = nc.dram_tensor("tri", [128, 128], F32, kind="ExternalInput").ap()
    out_d = nc.dram_tensor("out", [T, C], F16, kind="ExternalOutput").ap()

    with tile.TileContext(nc) as tc, ExitStack() as ctx:
        consts = ctx.enter_context(tc.tile_pool(name="consts", bufs=1))

        cc_sb = consts.tile([128, T], F16)
        ss_sb = consts.tile([128, T], F16)   # pre-shuffled+signed sin
        tri_sb = consts.tile([128, 128], F32)
        bqk_sb = consts.tile([128, 4], F32)
        x1 = consts.tile([1, T], F16)

        # rotated Q^T / K^T: [pair][half] tiles (Q pairs 0-1, K pairs 2-3)
        qkt = [[consts.tile([128, 1024], F16, name=f"qkt{p}_{h}")
                for h in range(2)] for p in range(4)]
        vp = [consts.tile([128, 4 * 65], F16, name=f"vp{i}") for i in range(16)]
        vview = [v.rearrange("p (h d) -> p h d", d=65) for v in vp]
        usc = [[consts.tile([128, 512], F16, name=f"usc{p}_{w}")
                for w in range(4)] for p in range(2)]
        wp_sb = [consts.tile([128, C], F16, name=f"wp{p}") for p in range(2)]
        xts = [consts.tile([128, T], F16, name=f"xt{j}") for j in range(8)]
        # packed weights: 8 (9 for V) row-chunks side by side in the free dim
        wq_sb = consts.tile([128, 8 * 256], F16)
        wk_sb = consts.tile([128, 8 * 256], F16)
        wv_sb = consts.tile([128, 9 * 256], F16)

        h0, h1 = slice(0, 1024), slice(1024, 2048)

        # ---------- input DMA: few big transfers, priority order ----------
        wv_src = wvT.rearrange("(a p) c -> p a c", p=128)
        wq_src = wqT.rearrange("(a p) c -> p a c", p=128)
        wk_src = wkT.rearrange("(a p) c -> p a c", p=128)
        wp_src = wpT.rearrange("(a p) c -> p a c", p=128)

        # gpsimd: memsets first (they gate the V units), then the library;
        # no DMAs here — SWDGE issue costs ~1.7us each.
        nc.gpsimd.memset(x1[:], 1.0)
        for i in range(16):
            nc.gpsimd.memset(vview[i][:, :, 64], 1.0)
        nc.gpsimd.load_library(library_config.attn)

        q0, q1 = slice(0, 512), slice(512, 1024)
        nc.sync.dma_start(wv_sb.rearrange("p (a c) -> p a c", c=256), wv_src)
        nc.scalar.dma_start(bqk_sb[:], bqk[:])
        nc.scalar.dma_start(cc_sb[:, q0], cc_d[:, q0])
        nc.scalar.dma_start(ss_sb[:, q0], ss_d[:, q0])
        nc.scalar.dma_start(wq_sb.rearrange("p (a c) -> p a c", c=256),
                            wq_src)
        for j in range(8):
            nc.sync.dma_start(xts[j][:, q0], xT[128 * j:128 * (j + 1), q0])
        nc.scalar.dma_start(tri_sb[:], tri_d[:])
        for j in range(8):
            nc.sync.dma_start(xts[j][:, q1], xT[128 * j:128 * (j + 1), q1])
        nc.scalar.dma_start(wk_sb.rearrange("p (a c) -> p a c", c=256),
                            wk_src)
        nc.scalar.dma_start(cc_sb[:, q1], cc_d[:, q1])
        nc.scalar.dma_start(ss_sb[:, q1], ss_d[:, q1])
        for j in range(4):
            nc.sync.dma_start(xts[j][:, h1], xT[128 * j:128 * (j + 1), h1])
        nc.scalar.dma_start(cc_sb[:, h1], cc_d[:, h1])
        nc.scalar.dma_start(ss_sb[:, h1], ss_d[:, h1])
        for j in range(4, 8):
            nc.sync.dma_start(xts[j][:, h1], xT[128 * j:128 * (j + 1), h1])
        nc.scalar.dma_start(wp_sb[0][:], wp_src[:, 0, :])
        nc.scalar.dma_start(wp_sb[1][:], wp_src[:, 1, :])

        # persistent SBUF pools
        rp = ctx.enter_context(tc.tile_pool(name="rope", bufs=3))
        epl = ctx.enter_context(tc.tile_pool(name="epool", bufs=6))
        zrp = ctx.enter_context(tc.tile_pool(name="zrpool", bufs=4))
        rzbp = ctx.enter_context(tc.tile_pool(name="rzbpool", bufs=4))
        ost = ctx.enter_context(tc.tile_pool(name="ostage", bufs=6))
        pb = ctx.enter_context(tc.tile_pool(name="pbank", bufs=1,
                                            space="PSUM"))

        # PSUM: pA/pB two-bank score tiles, y0..y3 single-bank accumulators
        def pair_bank(tag, name):
            return pb.tile([128, 1024], F32, tag=tag, name=name)

        def ybank(k, name):
            return pb.tile([128, 512], F32, tag=f"y{k}", name=name)

        # filler psum: rotate over the y-banks listed in fb_state
        fb_state = {"banks": (0, 1, 2, 3), "ctr": 0}

        def fslot(name):
            banks = fb_state["banks"]
            k = banks[fb_state["ctr"] % len(banks)]
            fb_state["ctr"] += 1
            return ybank(k, name)

        # ---------------- unit definitions ----------------
        rope_ctr = [0]

        def qk_unit(isk, ci, half, tg):
            """One 512-col projection+rope unit for Q/K chunk ci."""
            csl = slice(1024 * half + 512 * tg, 1024 * half + 512 * tg + 512)
            wsl = slice(512 * tg, 512 * tg + 512)
            wsb = wk_sb if isk else wq_sb
            ps = fslot(f"qk{int(isk)}_{ci}_{half}_{tg}")
            for j in range(8):
                nc.tensor.matmul(
                    ps[:], wsb[:, 256 * j + 128 * ci:256 * j + 128 * ci + 128],
                    xts[j][:, csl], start=(j == 0), stop=(j == 7))
            bcol = (2 if isk else 0) + ci
            pair = (2 if isk else 0) + ci
            bias = bqk_sb[:, bcol:bcol + 1]
            pb16 = rp.tile([128, 512], F16, tag="pb16")
            t1 = rp.tile([128, 512], F16, tag="t1")
            shf = rp.tile([128, 512], F16, tag="shf")
            t2 = rp.tile([128, 512], F16, tag="t2")
            # Alternate rope units between DVE and gpsimd so window tri-adds
            # aren't queued behind every rope chain on DVE.
            nc.scalar.activation(pb16[:], ps[:], AF.Identity, bias=bias)
            nc.vector.stream_shuffle(shf[:], pb16[:], SWAP_MASK)
            eng = nc.vector
            rope_ctr[0] += 1
            eng.tensor_mul(t1[:], pb16[:], cc_sb[:, csl])
            eng.tensor_mul(t2[:], shf[:], ss_sb[:, csl])
            eng.tensor_add(qkt[pair][half][:, wsl], t1[:], t2[:])

        def v_unit(i):
            """V' s-tile i: vraw = x_i^T @ Wv (+bias), copy into vp[i]."""
            ps = fslot(f"v{i}")
            tsl = slice(128 * i, 128 * (i + 1))
            for j in range(8):
                nc.tensor.matmul(ps[:, 0:256], xts[j][:, tsl],
                                 wv_sb[:, 256 * j:256 * (j + 1)],
                                 start=(j == 0), stop=False)
            nc.tensor.matmul(ps[:, 0:256], x1[:, tsl],
                             wv_sb[0:1, 2048:2304], start=False, stop=True)
            nc.scalar.activation(vview[i][:, :, 0:64], ps[:, 0:256], AF.Copy)

        yz_live = {}

        def score_round(pr, w, i, tag, pool):
            """Merged score round: both heads of pair pr, s-tile i, window w.
            Both heads land in the two banks of one [128,1024] tile; one
            merged exp (or two trimmed ones on deep-diagonal rounds)."""
            sub0 = max(0, 128 * i - 512 * w)
            kt = qkt[2 + pr][i // 8]
            qt = qkt[pr][w // 2]
            qsl = slice((512 * w) % 1024 + sub0, (512 * w) % 1024 + 512)
            ps = pair_bank(tag, f"s{pr}_{w}_{i}")
            for hs in range(2):
                rows = slice(64 * hs, 64 * (hs + 1))
                nc.tensor.matmul(
                    ps[:, 512 * hs + sub0:512 * (hs + 1)],
                    kt[rows, 128 * (i % 8):128 * (i % 8) + 128],
                    qt[rows, qsl], start=True, stop=True)
            if i >= 4 * w:
                for hs in range(2):
                    o = 512 * hs + sub0
                    nc.vector.tensor_add(ps[:, o:o + 128], ps[:, o:o + 128],
                                         tri_sb[:])
            et = pool.tile([128, 1024], F16, tag="e", name=f"e{pr}_{w}_{i}")
            if sub0 > 0:
                for hs in range(2):
                    o = 512 * hs + sub0
                    e = 512 * (hs + 1)
                    nc.scalar.activation(et[:, o:e], ps[:, o:e], AF.Exp,
                                         scale=0.125)
            else:
                nc.scalar.activation(et[:], ps[:], AF.Exp, scale=0.125)
            return et, sub0

        def attv_round(pr, w, i, et, sub0, ni):
            for hs in range(2):
                h = 2 * pr + hs
                if i == 0:
                    yz_live[h] = ybank(h, f"yz{h}_{w}")
                nc.tensor.matmul(
                    yz_live[h][0:65, sub0:512],
                    vp[i][:, 65 * h:65 * (h + 1)],
                    et[:, 512 * hs + sub0:512 * (hs + 1)],
                    start=(i == 0), stop=(i == ni - 1))

        def norm_pair(pr, w):
            """Normalize both heads of pair pr for window w into usc."""
            rz, rb = {}, {}
            zr = {}
            for hs in range(2):
                h = 2 * pr + hs
                # PSUM holds e10m23; the recip's bitwise seed needs IEEE fp32
                # bits, so bounce z through SBUF via the (idle) ACT engine.
                zr[hs] = zrp.tile([1, 512], F32, tag="zrow", name=f"zc{h}_{w}")
                nc.scalar.activation(zr[hs][:], yz_live[h][64:65, :], AF.Copy)
            for hs in range(2):
                h = 2 * pr + hs
                rz[hs] = zrp.tile([1, 512], F32, tag="rzr", name=f"rr{h}_{w}")
                nc.vector.reciprocal_approx_fast(rz[hs][:], zr[hs][:])
            for hs in range(2):
                h = 2 * pr + hs
                rb[hs] = rzbp.tile([64, 512], F32, tag="rzb",
                                   name=f"rb{h}_{w}")
                nc.gpsimd.partition_broadcast(rb[hs][:], rz[hs][:])
            for hs in range(2):
                h = 2 * pr + hs
                nc.vector.tensor_mul(usc[pr][w][64 * hs:64 * (hs + 1), :],
                                     yz_live[h][0:64, :], rb[hs][:])

        def proj_unit(tch, cg, psl, on_act=False):
            """Output projection for t-chunk tch, 512-col group cg."""
            w = tch // 4
            tsl = slice(128 * (tch % 4), 128 * (tch % 4) + 128)
            csl = slice(512 * cg, 512 * (cg + 1))
            for pq in range(2):
                nc.tensor.matmul(psl[:, 0:512], usc[pq][w][:, tsl],
                                 wp_sb[pq][:, csl],
                                 start=(pq == 0), stop=(pq == 1))
            st = ost.tile([128, 512], F16, tag="ost", name=f"st{tch}_{cg}")
            if on_act:
                nc.scalar.activation(st[:], psl[:, 0:512], AF.Copy)
            else:
                nc.vector.tensor_copy(st[:], psl[:, 0:512])
            nc.sync.dma_start(out_d[128 * tch:128 * tch + 128, csl], st[:])

        # ---------------- A: startup stream ----------------
        # v0-3 rotate y0..y3; the first two qk units go once more through
        # y0/y1 (their readers are long done before w0's yz claims them).
        fb_state["banks"] = (0, 1, 2, 3)
        for i in range(4):
            v_unit(i)
        qk_unit(False, 0, 0, 0)
        qk_unit(True, 0, 0, 0)

        # ---------------- B: w0/w1 per pair + filler ----------------
        def window(w, pr, fill, nfill_share):
            """Attention window w for pair pr; scores alternate pA/pB,
            attV lags LAG rounds; fillers popped between."""
            ni = 4 * w + 4
            nr = ni + LAG
            pend = deque()
            emitted = 0
            for r in range(nr):
                if r < ni:
                    et, sub0 = score_round(pr, w, r,
                                           "pA" if r % 2 == 0 else "pB", epl)
                    pend.append((r, et, sub0))
                while fill and emitted < (r + 1) * nfill_share // nr:
                    fill.popleft()()
                    emitted += 1
                if r >= LAG:
                    i, et, sub0 = pend.popleft()
                    attv_round(pr, w, i, et, sub0, ni)
            norm_pair(pr, w)

        fill = deque()
        fill.append(lambda: qk_unit(False, 1, 0, 0))
        fill.append(lambda: qk_unit(True, 1, 0, 0))
        for isk in (False, True):
            fill.append(lambda isk=isk: qk_unit(isk, 0, 0, 1))
        for i in range(4, 8):
            fill.append(lambda i=i: v_unit(i))
        for isk in (False, True):
            fill.append(lambda isk=isk: qk_unit(isk, 1, 0, 1))
        for isk in (False, True):
            for tg in range(2):
                fill.append(lambda isk=isk, tg=tg: qk_unit(isk, 0, 1, tg))
        for i in range(8, 12):
            fill.append(lambda i=i: v_unit(i))
        for isk in (False, True):
            for tg in range(2):
                fill.append(lambda isk=isk, tg=tg: qk_unit(isk, 1, 1, tg))
        for i in range(12, 16):
            fill.append(lambda i=i: v_unit(i))

        nf = len(fill)
        shares = [nf * 5 // 32, nf * 12 // 32, nf * 22 // 32, nf]
        prev = 0
        for wi, (w, pr) in enumerate([(0, 0), (0, 1), (1, 0), (1, 1)]):
            # fillers use the y-banks of the inactive pair
            fb_state["banks"] = (2, 3) if pr == 0 else (0, 1)
            cnt = shares[wi] - prev
            prev = shares[wi]
            sub = deque(fill.popleft() for _ in range(cnt))
            window(w, pr, sub, cnt)
            while sub:
                sub.popleft()()

        # ---------------- C: w2/w3 as per-pair windows + proj filler -------
        # proj t-chunks become available as the usc windows complete:
        # tch0-7 after B, tch8-11 after both w2 norms, tch12-15 at the end.
        def projf(tch, cg, on_act=False):
            return lambda: proj_unit(tch, cg, fslot(f"op{tch}_{cg}"),
                                     on_act=on_act)

        fill = deque(projf(tch, cg) for tch in range(4) for cg in range(2))
        fb_state["banks"] = (2, 3)
        window(2, 0, fill, len(fill))
        fill = deque(projf(tch, cg) for tch in range(4, 8) for cg in range(2))
        fb_state["banks"] = (0, 1)
        window(2, 1, fill, len(fill))

        fill = deque(projf(tch, cg, True) for tch in range(8, 10)
                     for cg in range(2))
        fb_state["banks"] = (2, 3)
        window(3, 0, fill, len(fill))
        fill = deque(projf(tch, cg, True) for tch in range(10, 12)
                     for cg in range(2))
        fb_state["banks"] = (0, 1)
        window(3, 1, fill, len(fill))

        # ---------------- E: tail projections on pA/pB halves -------------
        pr_slots = {"ctr": 0, "cur": {}}

        def pslot(name):
            k = pr_slots["ctr"] % 4
            pr_slots["ctr"] += 1
            tag = "pA" if k < 2 else "pB"
            half = k % 2
            if half == 0:
                pr_slots["cur"][tag] = pair_bank(tag, name)
            return pr_slots["cur"][tag][:, 512 * half:512 * (half + 1)]

        for tch in range(12, 16):
            for cg in range(2):
                proj_unit(tch, cg, pslot(f"op{tch}_{cg}"), on_act=True)

    nc.compile()
    return nc


_NC_CACHE = {}


def _get_nc():
    if "nc" not in _NC_CACHE:
        _NC_CACHE["nc"] = build_nc()
    return _NC_CACHE["nc"]


def make_in_map(core, x, Wq, bq, Wk, bk, Wv, bv, Wp, bp, rope_cache):
    b = core // 4
    hbase = (core % 4) * 4

    xTa = np.empty((C + 1, T), np.float16)
    xTa[:C] = np.asarray(x[b], np.float32).T
    xTa[C] = 1.0

    # packed channel order for Q/K: per head, two 32-row quadrants; each
    # quadrant holds [even ch 16q..16q+15 | odd ch 16q..16q+15] so the rope
    # partner swap is lane l -> (l+16)%32 inside every quadrant.
    perm = []
    for p in range(2):
        for hh in range(2):
            h = hbase + 2 * p + hh
            for q in range(2):
                perm += [h * HD + 2 * (16 * q + m) for m in range(16)]
                perm += [h * HD + 2 * (16 * q + m) + 1 for m in range(16)]
    perm = np.asarray(perm)

    wqTa = np.ascontiguousarray(
        np.asarray(Wq, np.float32)[perm, :].T).astype(np.float16)
    wkTa = np.ascontiguousarray(
        np.asarray(Wk, np.float32)[perm, :].T).astype(np.float16)

    chs = np.arange(hbase * HD, hbase * HD + 256)
    wvTa = np.zeros((C + 128, 256), np.float16)
    wvTa[:C] = np.asarray(Wv, np.float32)[chs, :].T
    wvTa[C] = np.asarray(bv, np.float32)[chs]
    wpTa = np.ascontiguousarray(
        np.asarray(Wp, np.float32)[:, chs].T).astype(np.float16)

    bqp = np.asarray(bq, np.float32)[perm].reshape(2, 128).T
    bkp = np.asarray(bk, np.float32)[perm].reshape(2, 128).T
    bqk_a = np.concatenate([bqp, bkp], axis=1)  # [128, 4]

    rc = np.asarray(rope_cache, np.float32)  # [T, 32, 2]
    r = np.arange(128)
    lane = r % 32
    quad = (r // 32) % 2
    m = 16 * quad + (lane % 16)  # rotation pair index per row
    sign = np.where(lane < 16, 1.0, -1.0).astype(np.float32)
    cc_a = np.ascontiguousarray(rc[:, m, 0].T).astype(np.float16)
    ss_raw = (rc[:, m, 1].T * sign[:, None]).astype(np.float16)
    # pre-shuffle sin rows so t2 = shuffle(pb) * ss_pre == shuffle(pb * ss)
    swap = np.asarray(SWAP_MASK)
    rows = np.arange(128)
    src = (rows // 32) * 32 + swap[rows % 32]
    ss_a = np.ascontiguousarray(ss_raw[src, :])

    sl, tl = np.arange(128)[:, None], np.arange(128)[None, :]
    tri_a = np.where(tl >= sl, 0.0, NEG).astype(np.float32)

    return dict(xT=xTa, wqT=wqTa, wkT=wkTa, wvT=wvTa, wpT=wpTa,
                bqk=bqk_a, cc=cc_a, ss=ss_a, tri=tri_a)


def kernel(x, Wq, bq, Wk, bk, Wv, bv, Wp, bp, rope_cache):
    global LAST_EXEC_NS, LAST_RESULTS
    args = (x, Wq, bq, Wk, bk, Wv, bv, Wp, bp, rope_cache)
    nc = _get_nc()
    in_maps = [make_in_map(c, *args) for c in range(NCORES)]
    r = None
    for attempt in range(4):
        try:
            r = run_bass_kernel_spmd(nc, in_maps, list(range(NCORES)))
            break
        except Exception:
            # transient NRT exec-unit errors recover on re-dispatch
            if attempt == 3:
                raise
            time.sleep(5.0 * (attempt + 1))
    LAST_EXEC_NS = r.exec_time_ns
    LAST_RESULTS = r
    out = np.zeros((2, T, C), np.float32)
    for core in range(NCORES):
        out[core // 4] += np.asarray(r.results[core]["out"], np.float32)
    out += np.asarray(bp, np.float32)[None, None, :]
    return out


# revision 25
# speedup vs baseline: 1.3529x; 1.1284x over previous
"""Trainium2 Bass kernel for causal self-attention with RoPE.

Problem: B=2, T=2048, C=1024, H=16 heads, hd=64, fp32 in/out, causal, rotary.

Sharding: 8 cores = 2 batches x 4 head-groups. Core c handles batch c//4 and
heads [4*(c%4), 4*(c%4)+4). Each core computes its heads' Q/K/V projections,
RoPE, causal attention, and a partial output projection over its 256 input
channels; the host sums the 4 partial projections per batch and adds bp.

v2 design notes (from the v1 trace: ACT/exp is the co-bottleneck with PE,
and the v1 tail serialized on norm chains):
 - The two heads of a pair write adjacent PSUM banks of one [128,1024] tile
   and a single EXP covers both (1147ns vs 2x720ns); deep-diagonal rounds
   use two trimmed exps instead.
 - PSUM tags: pA/pB = two [128,1024] score tiles (2 banks each), y0..y3 =
   per-head attV accumulators (64 y-rows + z-row). Attention windows run
   per-pair so the inactive pair's y-banks serve as filler/proj PSUM.
 - Rope: ACT does the bias-add + fp32->fp16 cast out of PSUM (Identity with
   per-partition bias); sin is pre-shuffled on the host so DVE does only
   mul/shuffle/mul/add on fp16.
 - All output projections run as PE filler inside the ACT-bound score phases
   or the attV-w3 phase; out-DMA goes per 512-col chunk after each cast.
 - Input DMA: few big multi-dim transfers (v1 startup was sync-issue bound),
   ordered so V-units (cheapest deps) start first; issued from 4 engines.
 - Norm chains: reciprocal reads z directly from PSUM, gpsimd broadcasts,
   DVE multiplies; per-head chains are emitted stage-interleaved.
"""

import time
from collections import deque
from contextlib import ExitStack

import numpy as np

import concourse.bass as bass
import concourse.tile as tile
from concourse import bacc, library_config, mybir
from concourse.bass_utils import run_bass_kernel_spmd

F32 = mybir.dt.float32
F16 = mybir.dt.float16

T = 2048
C = 1024
HD = 64
NCORES = 8
NEG = -1e10
LAG = 3
SWAP_MASK = list(range(16, 32)) + list(range(16))

AF = mybir.ActivationFunctionType
ALU = mybir.AluOpType

LAST_EXEC_NS = None
LAST_RESULTS = None


def build_nc():
    nc = bacc.Bacc("TRN2", target_bir_lowering=False, debug=False)

    xT = nc.dram_tensor("xT", [C + 1, T], F16, kind="ExternalInput").ap()
    wqT = nc.dram_tensor("wqT", [C, 256], F16, kind="ExternalInput").ap()
    wkT = nc.dram_tensor("wkT", [C, 256], F16, kind="ExternalInput").ap()
    wvT = nc.dram_tensor("wvT", [C + 128, 256], F16, kind="ExternalInput").ap()
    wpT = nc.dram_tensor("wpT", [256, C], F16, kind="ExternalInput").ap()
    bqk = nc.dram_tensor("bqk", [128, 4], F32, kind="ExternalInput").ap()
    cc_d = nc.dram_tensor("cc", [128, T], F16, kind="ExternalInput").ap()
    ss_d = nc.dram_tensor("ss", [128, T], F16, kind="ExternalInput").ap()
    tri_d = nc.dram_tensor("tri", [128, 128], F32, kind="ExternalInput").ap()
    out_d = nc.dram_tensor("out", [T, C], F16, kind="ExternalOutput").ap()

    with tile.TileContext(nc) as tc, ExitStack() as ctx:
        consts = ctx.enter_context(tc.tile_pool(name="consts", bufs=1))

        cc_sb = consts.tile([128, T], F16)
        ss_sb = consts.tile([128, T], F16)   # pre-shuffled+signed sin
        tri_sb = consts.tile([128, 128], F32)
        bqk_sb = consts.tile([128, 4], F32)
        x1 = consts.tile([1, T], F16)

        # rotated Q^T / K^T: [pair][half] tiles (Q pairs 0-1, K pairs 2-3)
        qkt = [[consts.tile([128, 1024], F16, name=f"qkt{p}_{h}")
                for h in range(2)] for p in range(4)]
        vp = [consts.tile([128, 4 * 65], F16, name=f"vp{i}") for i in range(16)]
        vview = [v.rearrange("p (h d) -> p h d", d=65) for v in vp]
        usc = [[consts.tile([128, 512], F16, name=f"usc{p}_{w}")
                for w in range(4)] for p in range(2)]
        wp_sb = [consts.tile([128, C], F16, name=f"wp{p}") for p in range(2)]
        xts = [consts.tile([128, T], F16, name=f"xt{j}") for j in range(8)]
        # packed weights: 8 (9 for V) row-chunks side by side in the free dim
        wq_sb = consts.tile([128, 8 * 256], F16)
        wk_sb = consts.tile([128, 8 * 256], F16)
        wv_sb = consts.tile([128, 9 * 256], F16)

        h0, h1 = slice(0, 1024), slice(1024, 2048)

        # ---------- input DMA: few big transfers, priority order ----------
        wv_src = wvT.rearrange("(a p) c -> p a c", p=128)
        wq_src = wqT.rearrange("(a p) c -> p a c", p=128)
        wk_src = wkT.rearrange("(a p) c -> p a c", p=128)
        wp_src = wpT.rearrange("(a p) c -> p a c", p=128)

        # gpsimd: memsets first (they gate the V units), then the library;
        # no DMAs here — SWDGE issue costs ~1.7us each.
        nc.gpsimd.memset(x1[:], 1.0)
        for i in range(16):
            nc.gpsimd.memset(vview[i][:, :, 64], 1.0)
        nc.gpsimd.load_library(library_config.attn)

        q0, q1 = slice(0, 512), slice(512, 1024)
        nc.sync.dma_start(wv_sb.rearrange("p (a c) -> p a c", c=256), wv_src)
        nc.scalar.dma_start(bqk_sb[:], bqk[:])
        nc.scalar.dma_start(cc_sb[:, q0], cc_d[:, q0])
        nc.scalar.dma_start(ss_sb[:, q0], ss_d[:, q0])
        nc.scalar.dma_start(wq_sb.rearrange("p (a c) -> p a c", c=256),
                            wq_src)
        for j in range(8):
            nc.sync.dma_start(xts[j][:, q0], xT[128 * j:128 * (j + 1), q0])
        nc.scalar.dma_start(tri_sb[:], tri_d[:])
        for j in range(8):
            nc.sync.dma_start(xts[j][:, q1], xT[128 * j:128 * (j + 1), q1])
        nc.scalar.dma_start(wk_sb.rearrange("p (a c) -> p a c", c=256),
                            wk_src)
        nc.scalar.dma_start(cc_sb[:, q1], cc_d[:, q1])
        nc.scalar.dma_start(ss_sb[:, q1], ss_d[:, q1])
        for j in range(4):
            nc.sync.dma_start(xts[j][:, h1], xT[128 * j:128 * (j + 1), h1])
        nc.scalar.dma_start(cc_sb[:, h1], cc_d[:, h1])
        nc.scalar.dma_start(ss_sb[:, h1], ss_d[:, h1])
        for j in range(4, 8):
            nc.sync.dma_start(xts[j][:, h1], xT[128 * j:128 * (j + 1), h1])
        nc.scalar.dma_start(wp_sb[0][:], wp_src[:, 0, :])
        nc.scalar.dma_start(wp_sb[1][:], wp_src[:, 1, :])

        # persistent SBUF pools
        rp = ctx.enter_context(tc.tile_pool(name="rope", bufs=3))
        epl = ctx.enter_context(tc.tile_pool(name="epool", bufs=6))
        zrp = ctx.enter_context(tc.tile_pool(name="zrpool", bufs=4))
        rzbp = ctx.enter_context(tc.tile_pool(name="rzbpool", bufs=4))
        ost = ctx.enter_context(tc.tile_pool(name="ostage", bufs=6))
        pb = ctx.enter_context(tc.tile_pool(name="pbank", bufs=1,
                                            space="PSUM"))

        # PSUM: pA/pB two-bank score tiles, y0..y3 single-bank accumulators
        def pair_bank(tag, name):
            return pb.tile([128, 1024], F32, tag=tag, name=name)

        def ybank(k, name):
            return pb.tile([128, 512], F32, tag=f"y{k}", name=name)

        # filler psum: rotate over the y-banks listed in fb_state
        fb_state = {"banks": (0, 1, 2, 3), "ctr": 0}

        def fslot(name):
            banks = fb_state["banks"]
            k = banks[fb_state["ctr"] % len(banks)]
            fb_state["ctr"] += 1
            return ybank(k, name)

        # ---------------- unit definitions ----------------
        rope_ctr = [0]

        def qk_unit(isk, ci, half, tg):
            """One 512-col projection+rope unit for Q/K chunk ci."""
            csl = slice(1024 * half + 512 * tg, 1024 * half + 512 * tg + 512)
            wsl = slice(512 * tg, 512 * tg + 512)
            wsb = wk_sb if isk else wq_sb
            ps = fslot(f"qk{int(isk)}_{ci}_{half}_{tg}")
            for j in range(8):
                nc.tensor.matmul(
                    ps[:], wsb[:, 256 * j + 128 * ci:256 * j + 128 * ci + 128],
                    xts[j][:, csl], start=(j == 0), stop=(j == 7))
            bcol = (2 if isk else 0) + ci
            pair = (2 if isk else 0) + ci
            bias = bqk_sb[:, bcol:bcol + 1]
            pb16 = rp.tile([128, 512], F16, tag="pb16")
            t1 = rp.tile([128, 512], F16, tag="t1")
            shf = rp.tile([128, 512], F16, tag="shf")
            t2 = rp.tile([128, 512], F16, tag="t2")
            # Alternate rope units between DVE and gpsimd so window tri-adds
            # aren't queued behind every rope chain on DVE.
            nc.scalar.activation(pb16[:], ps[:], AF.Identity, bias=bias)
            nc.vector.stream_shuffle(shf[:], pb16[:], SWAP_MASK)
            eng = nc.vector
            rope_ctr[0] += 1
            eng.tensor_mul(t1[:], pb16[:], cc_sb[:, csl])
            eng.tensor_mul(t2[:], shf[:], ss_sb[:, csl])
            eng.tensor_add(qkt[pair][half][:, wsl], t1[:], t2[:])

        def v_unit(i):
            """V' s-tile i: vraw = x_i^T @ Wv (+bias), copy into vp[i]."""
            ps = fslot(f"v{i}")
            tsl = slice(128 * i, 128 * (i + 1))
            for j in range(8):
                nc.tensor.matmul(ps[:, 0:256], xts[j][:, tsl],
                                 wv_sb[:, 256 * j:256 * (j + 1)],
                                 start=(j == 0), stop=False)
            nc.tensor.matmul(ps[:, 0:256], x1[:, tsl],
                             wv_sb[0:1, 2048:2304], start=False, stop=True)
            nc.scalar.activation(vview[i][:, :, 0:64], ps[:, 0:256], AF.Copy)

        def score_round(pr, w, i, tag, pool):
            """Merged score round: both heads of pair pr, s-tile i, window w.
            Both heads land in the two banks of one [128,1024] tile; one
            merged exp (or two trimmed ones on deep-diagonal rounds)."""
            sub0 = max(0, 128 * i - 512 * w)
            kt = qkt[2 + pr][i // 8]
            qt = qkt[pr][w // 2]
            qsl = slice((512 * w) % 1024 + sub0, (512 * w) % 1024 + 512)
            ps = pair_bank(tag, f"s{pr}_{w}_{i}")
            for hs in range(2):
                rows = slice(64 * hs, 64 * (hs + 1))
                nc.tensor.matmul(
                    ps[:, 512 * hs + sub0:512 * (hs + 1)],
                    kt[rows, 128 * (i % 8):128 * (i % 8) + 128],
                    qt[rows, qsl], start=True, stop=True)
            if i >= 4 * w:
                for hs in range(2):
                    o = 512 * hs + sub0
                    nc.vector.tensor_add(ps[:, o:o + 128], ps[:, o:o + 128],
                                         tri_sb[:])
            et = pool.tile([128, 1024], F16, tag="e", name=f"e{pr}_{w}_{i}")
            if sub0 > 0:
                for hs in range(2):
                    o = 512 * hs + sub0
                    e = 512 * (hs + 1)
                    nc.scalar.activation(et[:, o:e], ps[:, o:e], AF.Exp,
                                         scale=0.125)
            else:
                nc.scalar.activation(et[:], ps[:], AF.Exp, scale=0.125)
            return et, sub0

        def attv_round(pr, w, i, et, sub0, ni, yz):
            """yz: per-window dict hs -> ybank; all windows share y0/y1
            (consecutive windows alternate pairs, tails interleave safely)."""
            for hs in range(2):
                h = 2 * pr + hs
                if i == 0:
                    yz[hs] = ybank(hs, f"yz{h}_{w}")
                nc.tensor.matmul(
                    yz[hs][0:65, sub0:512],
                    vp[i][:, 65 * h:65 * (h + 1)],
                    et[:, 512 * hs + sub0:512 * (hs + 1)],
                    start=(i == 0), stop=(i == ni - 1))

        def norm_pair(pr, w, yz):
            """Normalize both heads of pair pr for window w into usc."""
            rz, rb, zr = {}, {}, {}
            for hs in range(2):
                h = 2 * pr + hs
                # PSUM holds e10m23; the recip's bitwise seed needs IEEE fp32
                # bits, so bounce z through SBUF via the (idle) ACT engine.
                zr[hs] = zrp.tile([1, 512], F32, tag="zrow", name=f"zc{h}_{w}")
                nc.scalar.activation(zr[hs][:], yz[hs][64:65, :], AF.Copy)
            for hs in range(2):
                h = 2 * pr + hs
                rz[hs] = zrp.tile([1, 512], F32, tag="rzr", name=f"rr{h}_{w}")
                nc.vector.reciprocal_approx_fast(rz[hs][:], zr[hs][:])
            for hs in range(2):
                h = 2 * pr + hs
                rb[hs] = rzbp.tile([64, 512], F32, tag="rzb",
                                   name=f"rb{h}_{w}")
                nc.gpsimd.partition_broadcast(rb[hs][:], rz[hs][:])
            for hs in range(2):
                h = 2 * pr + hs
                nc.vector.tensor_mul(usc[pr][w][64 * hs:64 * (hs + 1), :],
                                     yz[hs][0:64, :], rb[hs][:])

        def proj_unit(tch, cg, psl, on_act=False):
            """Output projection for t-chunk tch, 512-col group cg."""
            w = tch // 4
            tsl = slice(128 * (tch % 4), 128 * (tch % 4) + 128)
            csl = slice(512 * cg, 512 * (cg + 1))
            for pq in range(2):
                nc.tensor.matmul(psl[:, 0:512], usc[pq][w][:, tsl],
                                 wp_sb[pq][:, csl],
                                 start=(pq == 0), stop=(pq == 1))
            st = ost.tile([128, 512], F16, tag="ost", name=f"st{tch}_{cg}")
            if on_act:
                nc.scalar.activation(st[:], psl[:, 0:512], AF.Copy)
            else:
                nc.vector.tensor_copy(st[:], psl[:, 0:512])
            nc.sync.dma_start(out_d[128 * tch:128 * tch + 128, csl], st[:])

        # ---------------- A: startup stream ----------------
        # y2/y3 are the permanent filler/projection banks; y0/y1 are the
        # yz accumulators for every window.
        fb_state["banks"] = (2, 3)
        qk_unit(False, 0, 0, 0)
        qk_unit(True, 0, 0, 0)
        for i in range(4):
            v_unit(i)

        # ---------------- streamed windows ----------------
        # Each window's last LAG attV rounds + its norm run as closures
        # interleaved into the next window's first rounds, so the PE never
        # drains at a window boundary.  `late` fillers (which depend on the
        # previous window's norm) only pop after round LAG.
        tail_q = deque()
        sctr = [0]

        def window_stream(w, pr, fill, late=()):
            ni = 4 * w + 4
            late = deque(late)
            pend = deque()
            yz = {}
            emitted = 0
            nfl = len(fill)
            for r in range(ni):
                while fill and emitted < (r + 1) * nfl // ni:
                    fill.popleft()()
                    emitted += 1
                tag = "pA" if sctr[0] % 2 == 0 else "pB"
                sctr[0] += 1
                pend.append((r, *score_round(pr, w, r, tag, epl)))
                if tail_q:
                    tail_q.popleft()()
                if r >= LAG:
                    i, et, sub0 = pend.popleft()
                    attv_round(pr, w, i, et, sub0, ni, yz)
                if r > LAG and late:
                    late.popleft()()
            while fill:
                fill.popleft()()
            while tail_q:
                tail_q.popleft()()
            while late:
                late.popleft()()
            while pend:
                i, et, sub0 = pend[0]
                tail_q.append(
                    lambda i=i, et=et, sub0=sub0:
                    attv_round(pr, w, i, et, sub0, ni, yz))
                pend.popleft()
            tail_q.append(lambda: norm_pair(pr, w, yz))

        def qkf(isk, ci, half, tg):
            return lambda: qk_unit(isk, ci, half, tg)

        def vf(i):
            return lambda: v_unit(i)

        def projf(tch, cg, on_act=False):
            return lambda: proj_unit(tch, cg, fslot(f"op{tch}_{cg}"),
                                     on_act=on_act)

        window_stream(0, 0, deque([qkf(False, 1, 0, 0), qkf(True, 1, 0, 0)]))
        window_stream(0, 1, deque([qkf(False, 0, 0, 1), qkf(True, 0, 0, 1),
                                   vf(4)]))
        window_stream(1, 0, deque([qkf(False, 1, 0, 1), qkf(True, 1, 0, 1),
                                   vf(5), vf(6), vf(7),
                                   qkf(False, 0, 1, 0)]))
        window_stream(1, 1, deque([qkf(True, 0, 1, 0), qkf(False, 1, 1, 0),
                                   qkf(True, 1, 1, 0), vf(8), vf(9), vf(10)]))
        window_stream(2, 0, deque([qkf(False, 0, 1, 1), qkf(True, 0, 1, 1),
                                   vf(11), projf(0, 0), projf(0, 1),
                                   projf(1, 0), projf(1, 1),
                                   qkf(False, 1, 1, 1)]))
        window_stream(2, 1, deque([qkf(True, 1, 1, 1), vf(12), vf(13),
                                   vf(14), vf(15), projf(2, 0), projf(2, 1),
                                   projf(3, 0), projf(3, 1)]))
        window_stream(3, 0,
                      deque([projf(t, cg) for t in range(4, 8)
                             for cg in range(2)]),
                      late=[projf(8, 0, True), projf(8, 1, False)])
        window_stream(3, 1, deque(),
                      late=[projf(t, cg, cg == 0) for t in range(9, 12)
                            for cg in range(2)])
        while tail_q:
            tail_q.popleft()()

        # ---------------- E: tail projections, split accumulation ---------
        # pq=0 (pair-0 usc, ready one window earlier) runs while the last
        # norm chain completes; pq=1 + cast + DMA follow.
        pr_slots = {"ctr": 0, "cur": {}}

        def pslot(name):
            k = pr_slots["ctr"] % 4
            pr_slots["ctr"] += 1
            tag = "pA" if k < 2 else "pB"
            half = k % 2
            if half == 0:
                pr_slots["cur"][tag] = pair_bank(tag, name)
            return pr_slots["cur"][tag][:, 512 * half:512 * (half + 1)]

        for wave in (12, 14):
            slots = {}
            for tch in (wave, wave + 1):
                for cg in range(2):
                    psl = slots[(tch, cg)] = pslot(f"op{tch}_{cg}")
                    tsl = slice(128 * (tch % 4), 128 * (tch % 4) + 128)
                    nc.tensor.matmul(psl[:, 0:512], usc[0][3][:, tsl],
                                     wp_sb[0][:, 512 * cg:512 * (cg + 1)],
                                     start=True, stop=False)
            for k, ((tch, cg), psl) in enumerate(slots.items()):
                tsl = slice(128 * (tch % 4), 128 * (tch % 4) + 128)
                csl = slice(512 * cg, 512 * (cg + 1))
                nc.tensor.matmul(psl[:, 0:512], usc[1][3][:, tsl],
                                 wp_sb[1][:, csl], start=False, stop=True)
                st = ost.tile([128, 512], F16, tag="ost",
                              name=f"st{tch}_{cg}")
                if k % 2 == 0:
                    nc.scalar.activation(st[:], psl[:, 0:512], AF.Copy)
                else:
                    nc.vector.tensor_copy(st[:], psl[:, 0:512])
                nc.sync.dma_start(out_d[128 * tch:128 * tch + 128, csl],
                                  st[:])

    nc.compile()
    return nc


_NC_CACHE = {}


def _get_nc():
    if "nc" not in _NC_CACHE:
        _NC_CACHE["nc"] = build_nc()
    return _NC_CACHE["nc"]


def make_in_map(core, x, Wq, bq, Wk, bk, Wv, bv, Wp, bp, rope_cache):
    b = core // 4
    hbase = (core % 4) * 4

    xTa = np.empty((C + 1, T), np.float16)
    xTa[:C] = np.asarray(x[b], np.float32).T
    xTa[C] = 1.0

    # packed channel order for Q/K: per head, two 32-row quadrants; each
    # quadrant holds [even ch 16q..16q+15 | odd ch 16q..16q+15] so the rope
    # partner swap is lane l -> (l+16)%32 inside every quadrant.
    perm = []
    for p in range(2):
        for hh in range(2):
            h = hbase + 2 * p + hh
            for q in range(2):
                perm += [h * HD + 2 * (16 * q + m) for m in range(16)]
                perm += [h * HD + 2 * (16 * q + m) + 1 for m in range(16)]
    perm = np.asarray(perm)

    wqTa = np.ascontiguousarray(
        np.asarray(Wq, np.float32)[perm, :].T).astype(np.float16)
    wkTa = np.ascontiguousarray(
        np.asarray(Wk, np.float32)[perm, :].T).astype(np.float16)

    chs = np.arange(hbase * HD, hbase * HD + 256)
    wvTa = np.zeros((C + 128, 256), np.float16)
    wvTa[:C] = np.asarray(Wv, np.float32)[chs, :].T
    wvTa[C] = np.asarray(bv, np.float32)[chs]
    wpTa = np.ascontiguousarray(
        np.asarray(Wp, np.float32)[:, chs].T).astype(np.float16)

    bqp = np.asarray(bq, np.float32)[perm].reshape(2, 128).T
    bkp = np.asarray(bk, np.float32)[perm].reshape(2, 128).T
    bqk_a = np.concatenate([bqp, bkp], axis=1)  # [128, 4]

    rc = np.asarray(rope_cache, np.float32)  # [T, 32, 2]
    r = np.arange(128)
    lane = r % 32
    quad = (r // 32) % 2
    m = 16 * quad + (lane % 16)  # rotation pair index per row
    sign = np.where(lane < 16, 1.0, -1.0).astype(np.float32)
    cc_a = np.ascontiguousarray(rc[:, m, 0].T).astype(np.float16)
    ss_raw = (rc[:, m, 1].T * sign[:, None]).astype(np.float16)
    # pre-shuffle sin rows so t2 = shuffle(pb) * ss_pre == shuffle(pb * ss)
    swap = np.asarray(SWAP_MASK)
    rows = np.arange(128)
    src = (rows // 32) * 32 + swap[rows % 32]
    ss_a = np.ascontiguousarray(ss_raw[src, :])

    sl, tl = np.arange(128)[:, None], np.arange(128)[None, :]
    tri_a = np.where(tl >= sl, 0.0, NEG).astype(np.float32)

    return dict(xT=xTa, wqT=wqTa, wkT=wkTa, wvT=wvTa, wpT=wpTa,
                bqk=bqk_a, cc=cc_a, ss=ss_a, tri=tri_a)


def kernel(x, Wq, bq, Wk, bk, Wv, bv, Wp, bp, rope_cache):
    global LAST_EXEC_NS, LAST_RESULTS
    args = (x, Wq, bq, Wk, bk, Wv, bv, Wp, bp, rope_cache)
    nc = _get_nc()
    in_maps = [make_in_map(c, *args) for c in range(NCORES)]
    r = None
    for attempt in range(4):
        try:
            r = run_bass_kernel_spmd(nc, in_maps, list(range(NCORES)))
            break
        except Exception:
            # transient NRT exec-unit errors recover on re-dispatch
            if attempt == 3:
                raise
            time.sleep(5.0 * (attempt + 1))
    LAST_EXEC_NS = r.exec_time_ns
    LAST_RESULTS = r
    out = np.zeros((2, T, C), np.float32)
    for core in range(NCORES):
        out[core // 4] += np.asarray(r.results[core]["out"], np.float32)
    out += np.asarray(bp, np.float32)[None, None, :]
    return out
